# revision 9
# baseline (speedup 1.0000x reference)
"""Causal self-attention (B=2, T=2048, C=1024, H=16) on 8 TRN2 NeuronCores.

Sharding (Megatron-style, per the hint): each core owns one PAIR of heads
(2c, 2c+1) for BOTH batches.  Column-sharded W_qkv produces qT/kT/vT in
[feature, token] layout (the host feeds x pre-transposed so contraction is
always over partitions); v is re-laid-out to natural [token, d] via PE
transposes.  Attention computes S^T = k q^T with the two heads row-packed
in the PE array (K=64 each at partition offsets 0/64), exp on ACT with
the 1/sqrt(D) scale folded in (ACT runs exp ONLY, so its spline table is
loaded once), causal masking via host-precomputed multiplicative masks on
the diagonal tiles of each q-chunk, and A@V with a ones-column appended
to v (M=65) so the softmax denominators fall out of the same matmul.
All biases are added with K=1 rank-1 matmuls into PSUM; all PSUM->SBUF
copies run on DVE.  An 8-core AllToAll swaps head-shards for
token-shards, after which each core computes its [512, 1024] slab of the
output projection with the full (replicated) W_proj.  The host only
shards/transposes/casts inputs and concatenates the 8 output slabs.

Compute dtype bf16 (f32 accumulation in PSUM); I/O f32.
"""

import os
import sys
import types

import numpy as np

if "/opt/trn_rl_repo" not in sys.path:
    sys.path.insert(0, "/opt/trn_rl_repo")

# antenv.axon_hooks is missing on this image; shim it so trace=True can
# capture NTFF profiles (used by test harnesses; harmless otherwise).
if "antenv.axon_hooks" not in sys.modules:
    _hooks_mod = types.ModuleType("antenv.axon_hooks")
    _holder = {"hook": None}
    _hooks_mod.set_axon_ntff_profile_hook = lambda h: _holder.__setitem__("hook", h)
    _hooks_mod.get_axon_ntff_profile_hook = lambda: _holder["hook"]
    sys.modules["antenv.axon_hooks"] = _hooks_mod
    try:
        from trn_agent_boot.trn_boot import _ntff_profile_via_ctypes

        _hooks_mod.set_axon_ntff_profile_hook(
            _ntff_profile_via_ctypes("/opt/axon/libaxon_pjrt.so")
        )
    except Exception:
        pass

import ml_dtypes
from contextlib import ExitStack

import concourse.bacc as bacc
import concourse.tile as tile
from concourse import mybir
from concourse.bass_utils import run_bass_kernel_spmd

B, T, C, H = 2, 2048, 1024, 16
D = C // H          # 64
NCORES = 8
HP = 2              # heads per core
TT = B * T          # 4096 global (b, t) rows
NK = C // 128       # 8 contraction tiles over features
NW = TT // 512      # 8 token windows
NQ = T // 512       # 4 q-chunks per batch
SHARD = TT // NCORES  # 512 output rows per core

F32 = mybir.dt.float32
BF = mybir.dt.bfloat16

ActF = mybir.ActivationFunctionType

_CACHE = {}

LAST_EXEC_TIME_NS = None
LAST_RESULTS = None


def build_nc():
    nc = bacc.Bacc("TRN2", target_bir_lowering=False, debug=False,
                   num_devices=NCORES)

    xT = nc.declare_dram_parameter("xT", [C, TT], BF, isOutput=False)
    wqkv = nc.declare_dram_parameter("wqkv", [C, 3 * 128], BF, isOutput=False)
    wproj = nc.declare_dram_parameter("wproj", [C, C], BF, isOutput=False)
    masks = nc.declare_dram_parameter("masks", [128, 4 * 512], BF, isOutput=False)
    brows = nc.declare_dram_parameter("brows", [1, 3 * 128 + C], BF, isOutput=False)
    ident = nc.declare_dram_parameter("ident", [128, 128], BF, isOutput=False)
    out = nc.declare_dram_parameter("out", [SHARD, C], F32, isOutput=True)

    with tile.TileContext(nc) as tc, ExitStack() as ctx:
        sb_x = ctx.enter_context(tc.tile_pool(name="sb_x", bufs=2))
        sb_w = ctx.enter_context(tc.tile_pool(name="sb_w", bufs=1))
        sb_qk = ctx.enter_context(tc.tile_pool(name="sb_qk", bufs=1))
        sb_v = ctx.enter_context(tc.tile_pool(name="sb_v", bufs=1))
        sb_att = ctx.enter_context(tc.tile_pool(name="sb_att", bufs=2))
        sb_y = ctx.enter_context(tc.tile_pool(name="sb_y", bufs=1))
        sb_tmp = ctx.enter_context(tc.tile_pool(name="sb_tmp", bufs=2))
        sb_out = ctx.enter_context(tc.tile_pool(name="sb_out", bufs=2))
        ps_mm = ctx.enter_context(tc.tile_pool(name="ps_mm", bufs=2, space="PSUM"))
        ps_s = ctx.enter_context(tc.tile_pool(name="ps_s", bufs=1, space="PSUM"))
        ps_y = ctx.enter_context(tc.tile_pool(name="ps_y", bufs=2, space="PSUM"))
        dram = ctx.enter_context(tc.tile_pool(name="dram", bufs=1, space="DRAM"))

        # ---- small loads first (weights, masks, biases) ----
        wqkv_sb = []
        for kk in range(NK):
            t = sb_w.tile([128, 3 * 128], BF, tag=f"wqkv{kk}")
            nc.gpsimd.dma_start(t[:], wqkv[128 * kk:128 * (kk + 1), :])
            wqkv_sb.append(t)
        mask_sb = sb_w.tile([128, 4 * 512], BF, tag="mask")
        nc.gpsimd.dma_start(mask_sb[:], masks[:])
        brows_sb = sb_w.tile([1, 3 * 128 + C], BF, tag="brows")
        nc.gpsimd.dma_start(brows_sb[:], brows[:])
        ident_sb = sb_w.tile([128, 128], BF, tag="ident")
        nc.gpsimd.dma_start(ident_sb[:], ident[:])

        ones_sb = sb_w.tile([1, 512], BF, tag="ones")
        nc.vector.memset(ones_sb[:], 1.0)

        # qT/kT/vT: [128 (=2 heads x 64 features), 4096 tokens]
        qT_sb = sb_qk.tile([128, TT], BF, tag="qT")
        kT_sb = sb_qk.tile([128, TT], BF, tag="kT")
        vT_sb = sb_qk.tile([128, TT], BF, tag="vT")
        dests = [qT_sb, kT_sb, vT_sb]
        # v natural: 32 tiles [128 tokens, 130] = [vA(64) | 1 | vB(64) | 1]
        v_sb = [sb_v.tile([128, 130], BF, tag=f"v{tt}", name=f"v{tt}")
                for tt in range(TT // 128)]

        # ---- QKV projection, streamed over token windows ----
        for n in range(NW):
            xw = []
            for kk in range(NK):
                t = sb_x.tile([128, 512], BF, tag=f"xw{kk}")
                nc.gpsimd.dma_start(t[:], xT[128 * kk:128 * (kk + 1),
                                             512 * n:512 * (n + 1)])
                xw.append(t)
            for m in range(3):
                ps = ps_mm.tile([128, 512], F32, tag="mm")
                for kk in range(NK):
                    nc.tensor.matmul(
                        ps[:], wqkv_sb[kk][:, 128 * m:128 * (m + 1)], xw[kk][:],
                        start=(kk == 0), stop=False)
                # bias over partitions: ps[p, t] += b[p] * 1
                nc.tensor.matmul(ps[:], brows_sb[:, 128 * m:128 * (m + 1)],
                                 ones_sb[:], start=False, stop=True)
                nc.vector.tensor_copy(dests[m][:, 512 * n:512 * (n + 1)], ps[:])
            # transpose v window into natural layout
            for tt in range(4 * n, 4 * (n + 1)):
                tp = ps_mm.tile([128, 128], BF, tag="mm")
                nc.tensor.transpose(tp[:], vT_sb[:, 128 * tt:128 * (tt + 1)],
                                    ident_sb[:])
                vt = v_sb[tt]
                nc.vector.memset(vt[:], 1.0)
                nc.vector.tensor_copy(
                    vt[:].rearrange("p (h c) -> p h c", c=65)[:, :, 0:64],
                    tp[:].rearrange("p (h c) -> p h c", c=64))

        # ---- attention ----
        yT_sb = sb_y.tile([128, TT], BF, tag="yT")

        for b in range(B):
            tb = b * T
            for j in range(NQ):
                kmax = 4 * (j + 1)
                qsl = slice(tb + 512 * j, tb + 512 * (j + 1))
                # attT: per head `kmax` slots of [128 kpos, 512 q];
                # head A at cols 0:8192, head B at cols 8192:16384
                attT = sb_att.tile([128, 2 * 16 * 512], BF, tag="attT")
                # S^T in groups of 2 k-tiles x 2 heads -> one psum tile
                for g in range(kmax // 2):
                    sps = ps_s.tile([128, 2048], F32, tag="s")
                    for i in range(2):
                        kt = 2 * g + i
                        ksl = slice(tb + 128 * kt, tb + 128 * (kt + 1))
                        for h in range(2):
                            hsl = slice(64 * h, 64 * (h + 1))
                            nc.tensor.matmul(
                                sps[:, 1024 * h + 512 * i:1024 * h + 512 * (i + 1)],
                                kT_sb[hsl, ksl], qT_sb[hsl, qsl],
                                start=True, stop=True)
                    # exp over the whole group (both heads)
                    dst = attT[:].rearrange("p (h s) -> p h s", h=2)[
                        :, :, 512 * 2 * g:512 * 2 * (g + 1)]
                    nc.scalar.activation(dst, sps[:].rearrange(
                        "p (h s) -> p h s", h=2), ActF.Exp,
                        scale=float(1.0 / np.sqrt(D)))
                # causal masks on the 4 diagonal k-tiles
                for i in range(4):
                    kt = 4 * j + i
                    for h in range(2):
                        a = attT[:, 8192 * h + 512 * kt:8192 * h + 512 * (kt + 1)]
                        nc.vector.tensor_mul(a, a, mask_sb[:, 512 * i:512 * (i + 1)])
                # A @ V (ones column gives the softmax denominator in row 64)
                for h in range(2):
                    yps = ps_y.tile([65, 512], F32, tag="y")
                    for kt in range(kmax):
                        nc.tensor.matmul(
                            yps[:], v_sb[b * 16 + kt][:, 65 * h:65 * (h + 1)],
                            attT[:, 8192 * h + 512 * kt:8192 * h + 512 * (kt + 1)],
                            start=(kt == 0), stop=(kt == kmax - 1))
                    ysb = sb_tmp.tile([65, 512], F32, tag="ysb")
                    nc.vector.tensor_copy(ysb[:], yps[:])
                    ltmp = sb_tmp.tile([1, 512], F32, tag="ltmp")
                    nc.vector.tensor_copy(ltmp[:], ysb[64:65, :])
                    recf = sb_tmp.tile([1, 512], F32, tag="recf")
                    nc.vector.reciprocal_approx_fast(recf[:], ltmp[:])
                    rec = sb_tmp.tile([1, 512], BF, tag="rec")
                    nc.vector.tensor_copy(rec[:], recf[:])
                    bc = ps_mm.tile([64, 512], F32, tag="mm")
                    nc.tensor.matmul(bc[:], ones_sb[:, 0:64], rec[:],
                                     start=True, stop=True)
                    nc.vector.tensor_mul(
                        yT_sb[64 * h:64 * (h + 1), qsl], ysb[0:64, :], bc[:])

        # ---- AllToAll: head-shard -> token-shard ----
        cc_in = dram.tile([NCORES * 128, 512], BF, tag="ccin")
        cc_out = dram.tile([NCORES * 128, 512], BF, tag="ccout")
        for sh in range(NCORES):
            nc.gpsimd.dma_start(cc_in[128 * sh:128 * (sh + 1), :],
                                yT_sb[:, 512 * sh:512 * (sh + 1)])
        nc.gpsimd.collective_compute(
            "AllToAll", mybir.AluOpType.bypass,
            replica_groups=[list(range(NCORES))],
            ins=[cc_in[:]], outs=[cc_out[:]])

        y_lhs = []
        for kk in range(NK):
            t = sb_tmp.tile([128, 512], BF, tag=f"ylhs{kk}")
            nc.gpsimd.dma_start(t[:], cc_out[128 * kk:128 * (kk + 1), :])
            y_lhs.append(t)

        # wproj is only needed at the end; let its DMA fill idle mid-kernel
        # bandwidth (emitted late so it never delays the x/w/mask loads).
        wproj_sb = []
        for kk in range(NK):
            t = sb_w.tile([128, C], BF, tag=f"wproj{kk}")
            nc.gpsimd.dma_start(t[:], wproj[128 * kk:128 * (kk + 1), :])
            wproj_sb.append(t)

        # ---- output projection: out[512, 1024] ----
        for mt in range(SHARD // 128):
            for nn in range(C // 512):
                ps = ps_mm.tile([128, 512], F32, tag="mm")
                for kk in range(NK):
                    nc.tensor.matmul(
                        ps[:], y_lhs[kk][:, 128 * mt:128 * (mt + 1)],
                        wproj_sb[kk][:, 512 * nn:512 * (nn + 1)],
                        start=(kk == 0), stop=False)
                nc.tensor.matmul(
                    ps[:], ones_sb[:, 0:128],
                    brows_sb[:, 384 + 512 * nn:384 + 512 * (nn + 1)],
                    start=False, stop=True)
                o = sb_out.tile([128, 512], F32, tag="o")
                nc.vector.tensor_copy(o[:], ps[:])
                nc.gpsimd.dma_start(
                    out[128 * mt:128 * (mt + 1), 512 * nn:512 * (nn + 1)], o[:])

    nc.compile()
    return nc


def _host_inputs(x, W_qkv, b_qkv, W_proj, b_proj):
    """Shard/layout/cast inputs for each core."""
    bf = ml_dtypes.bfloat16
    xT = np.ascontiguousarray(
        x.reshape(TT, C).T).astype(bf)                    # [C, TT]
    wproj = W_proj.astype(bf)                             # [C, C]
    kk_idx = np.arange(128)[:, None]
    qq_idx = np.arange(512)[None, :]
    masks = np.concatenate(
        [(128 * i + kk_idx <= qq_idx) for i in range(4)],
        axis=1).astype(bf)                                # [128, 2048]
    ident = np.eye(128).astype(bf)

    in_maps = []
    for c in range(NCORES):
        h0 = HP * c * D
        cols = slice(h0, h0 + HP * D)                     # 128 cols
        wq = W_qkv[:, cols]
        wk = W_qkv[:, C:][:, cols]
        wv = W_qkv[:, 2 * C:][:, cols]
        wqkv = np.concatenate([wq, wk, wv], axis=1).astype(bf)   # [C, 384]
        brows = np.concatenate(
            [b_qkv[cols], b_qkv[C:][cols], b_qkv[2 * C:][cols], b_proj]
        )[None, :].astype(bf)                             # [1, 1408]
        in_maps.append({
            "xT": xT, "wqkv": wqkv, "wproj": wproj,
            "masks": masks, "brows": brows, "ident": ident,
        })
    return in_maps


def kernel(x, W_qkv, b_qkv, W_proj, b_proj):
    global LAST_EXEC_TIME_NS, LAST_RESULTS
    x = np.asarray(x, dtype=np.float32)
    W_qkv = np.asarray(W_qkv, dtype=np.float32)
    b_qkv = np.asarray(b_qkv, dtype=np.float32)
    W_proj = np.asarray(W_proj, dtype=np.float32)
    b_proj = np.asarray(b_proj, dtype=np.float32)

    if "nc" not in _CACHE:
        _CACHE["nc"] = build_nc()
    nc = _CACHE["nc"]

    in_maps = _host_inputs(x, W_qkv, b_qkv, W_proj, b_proj)
    trace = os.environ.get("TRN_KERNEL_TRACE", "0") == "1"
    res = run_bass_kernel_spmd(nc, in_maps, core_ids=list(range(NCORES)),
                               trace=trace)
    LAST_EXEC_TIME_NS = res.exec_time_ns
    LAST_RESULTS = res
    out = np.concatenate([res.results[c]["out"] for c in range(NCORES)],
                         axis=0)
    return out.reshape(B, T, C).astype(np.float32)


# revision 13
# speedup vs baseline: 1.0844x; 1.0844x over previous
"""Causal self-attention (B=2, T=2048, C=1024, H=16) on 8 TRN2 NeuronCores.

Sharding (Megatron-style, per the hint): each core owns one PAIR of heads
(2c, 2c+1) for BOTH batches.  Column-sharded W_qkv produces qT/kT/vT in
[feature, token] layout (the host feeds x pre-transposed so contraction is
always over partitions); v is re-laid-out to natural [token, d] via PE
transposes.  Attention computes S^T = k q^T with the two heads row-packed
in the PE array (K=64 each at partition offsets 0/64), exp on ACT with
the 1/sqrt(D) scale folded in (ACT runs exp ONLY, so its spline table is
loaded once), causal masking via host-precomputed multiplicative masks on
the diagonal tiles of each q-chunk, and A@V with a ones-column appended
to v (M=65) so the softmax denominators fall out of the same matmul.
All biases are added with K=1 rank-1 matmuls into PSUM; all PSUM->SBUF
copies run on DVE.  An 8-core AllToAll swaps head-shards for
token-shards, after which each core computes its [512, 1024] slab of the
output projection with the full (replicated) W_proj.  The host only
shards/transposes/casts inputs and concatenates the 8 output slabs.

Compute dtype bf16 (f32 accumulation in PSUM); I/O f32.
"""

import os
import sys
import types

import numpy as np

if "/opt/trn_rl_repo" not in sys.path:
    sys.path.insert(0, "/opt/trn_rl_repo")

# antenv.axon_hooks is missing on this image; shim it so trace=True can
# capture NTFF profiles (used by test harnesses; harmless otherwise).
if "antenv.axon_hooks" not in sys.modules:
    _hooks_mod = types.ModuleType("antenv.axon_hooks")
    _holder = {"hook": None}
    _hooks_mod.set_axon_ntff_profile_hook = lambda h: _holder.__setitem__("hook", h)
    _hooks_mod.get_axon_ntff_profile_hook = lambda: _holder["hook"]
    sys.modules["antenv.axon_hooks"] = _hooks_mod
    try:
        from trn_agent_boot.trn_boot import _ntff_profile_via_ctypes

        _hooks_mod.set_axon_ntff_profile_hook(
            _ntff_profile_via_ctypes("/opt/axon/libaxon_pjrt.so")
        )
    except Exception:
        pass

import ml_dtypes
from contextlib import ExitStack

import concourse.bacc as bacc
import concourse.tile as tile
from concourse import mybir
from concourse.bass_utils import run_bass_kernel_spmd

B, T, C, H = 2, 2048, 1024, 16
D = C // H          # 64
NCORES = 8
HP = 2              # heads per core
TT = B * T          # 4096 global (b, t) rows
NK = C // 128       # 8 contraction tiles over features
NW = TT // 512      # 8 token windows
NQ = T // 512       # 4 q-chunks per batch
SHARD = TT // NCORES  # 512 output rows per core

F32 = mybir.dt.float32
BF = mybir.dt.bfloat16

ActF = mybir.ActivationFunctionType

_CACHE = {}

LAST_EXEC_TIME_NS = None
LAST_RESULTS = None


def build_nc():
    nc = bacc.Bacc("TRN2", target_bir_lowering=False, debug=False,
                   num_devices=NCORES)

    xT = nc.declare_dram_parameter("xT", [C, TT], BF, isOutput=False)
    wqkv = nc.declare_dram_parameter("wqkv", [C, 3 * 128], BF, isOutput=False)
    wproj = nc.declare_dram_parameter("wproj", [C, C], BF, isOutput=False)
    masks = nc.declare_dram_parameter("masks", [128, 4 * 512], BF, isOutput=False)
    brows = nc.declare_dram_parameter("brows", [1, 3 * 128 + C], BF, isOutput=False)
    ident = nc.declare_dram_parameter("ident", [128, 128], BF, isOutput=False)
    out = nc.declare_dram_parameter("out", [SHARD, C], F32, isOutput=True)

    with tile.TileContext(nc) as tc, ExitStack() as ctx:
        sb_x = ctx.enter_context(tc.tile_pool(name="sb_x", bufs=2))
        sb_w = ctx.enter_context(tc.tile_pool(name="sb_w", bufs=1))
        sb_qk = ctx.enter_context(tc.tile_pool(name="sb_qk", bufs=1))
        sb_v = ctx.enter_context(tc.tile_pool(name="sb_v", bufs=1))
        sb_att = ctx.enter_context(tc.tile_pool(name="sb_att", bufs=1))
        sb_y = ctx.enter_context(tc.tile_pool(name="sb_y", bufs=1))
        sb_tmp = ctx.enter_context(tc.tile_pool(name="sb_tmp", bufs=2))
        sb_out = ctx.enter_context(tc.tile_pool(name="sb_out", bufs=2))
        ps_mm = ctx.enter_context(tc.tile_pool(name="ps_mm", bufs=2, space="PSUM"))
        ps_s = ctx.enter_context(tc.tile_pool(name="ps_s", bufs=1, space="PSUM"))
        ps_y = ctx.enter_context(tc.tile_pool(name="ps_y", bufs=2, space="PSUM"))
        dram = ctx.enter_context(tc.tile_pool(name="dram", bufs=1, space="DRAM"))

        # ---- small loads first (weights, masks, biases) ----
        wqkv_sb = []
        for kk in range(NK):
            t = sb_w.tile([128, 3 * 128], BF, tag=f"wqkv{kk}")
            nc.gpsimd.dma_start(t[:], wqkv[128 * kk:128 * (kk + 1), :])
            wqkv_sb.append(t)
        mask_sb = sb_w.tile([128, 4 * 512], BF, tag="mask")
        nc.gpsimd.dma_start(mask_sb[:], masks[:])
        brows_sb = sb_w.tile([1, 3 * 128 + C], BF, tag="brows")
        nc.gpsimd.dma_start(brows_sb[:], brows[:])
        ident_sb = sb_w.tile([128, 128], BF, tag="ident")
        nc.gpsimd.dma_start(ident_sb[:], ident[:])

        ones_sb = sb_w.tile([1, 512], BF, tag="ones")
        nc.vector.memset(ones_sb[:], 1.0)

        # qT/kT/vT: [128 (=2 heads x 64 features), 4096 tokens]
        qT_sb = sb_qk.tile([128, TT], BF, tag="qT")
        kT_sb = sb_qk.tile([128, TT], BF, tag="kT")
        vT_sb = sb_qk.tile([128, TT], BF, tag="vT")
        dests = [qT_sb, kT_sb, vT_sb]
        # v natural: [128 tokens, 32 tiles x 130] = [vA(64) | 1 | vB(64) | 1]
        v_all = sb_v.tile([128, (TT // 128) * 130], BF, tag="v")
        v_sb = [v_all[:, 130 * tt:130 * (tt + 1)] for tt in range(TT // 128)]

        # ---- QKV projection, streamed over token windows ----
        def qkv_window(n):
            xw = []
            for kk in range(NK):
                t = sb_x.tile([128, 512], BF, tag=f"xw{kk}", name=f"xw{kk}_{n}")
                nc.gpsimd.dma_start(t[:], xT[128 * kk:128 * (kk + 1),
                                             512 * n:512 * (n + 1)])
                xw.append(t)
            for m in range(3):
                ps = ps_mm.tile([128, 512], F32, tag="mm", name=f"qkvps{n}_{m}")
                for kk in range(NK):
                    nc.tensor.matmul(
                        ps[:], wqkv_sb[kk][:, 128 * m:128 * (m + 1)], xw[kk][:],
                        start=(kk == 0), stop=False)
                # bias over partitions: ps[p, t] += b[p] * 1
                nc.tensor.matmul(ps[:], brows_sb[:, 128 * m:128 * (m + 1)],
                                 ones_sb[:], start=False, stop=True)
                nc.vector.tensor_copy(dests[m][:, 512 * n:512 * (n + 1)], ps[:])
            # transpose v window into natural layout
            for tt in range(4 * n, 4 * (n + 1)):
                tp = ps_mm.tile([128, 128], BF, tag="mm", name=f"vtp{tt}")
                nc.tensor.transpose(tp[:], vT_sb[:, 128 * tt:128 * (tt + 1)],
                                    ident_sb[:])
                vt = v_sb[tt]
                nc.vector.memset(vt, 1.0)
                nc.vector.tensor_copy(
                    vt.rearrange("p (h c) -> p h c", c=65)[:, :, 0:64],
                    tp[:].rearrange("p (h c) -> p h c", c=64))

        # ---- attention ----
        yT_sb = sb_y.tile([128, TT], BF, tag="yT")
        # two attT tiles, alternated across chunks; memset once so that the
        # exp-skipped (causally invalid) columns of diagonal tiles hold
        # finite stale data for the mask-multiply to zero.
        attT_tiles = []
        for i in range(2):
            t = sb_att.tile([128, 2 * 16 * 512], BF, tag=f"attT{i}", name=f"attT{i}")
            nc.vector.memset(t[:], 0.0)
            attT_tiles.append(t)

        def attention_chunk(b, j):
            tb = b * T
            kmax = 4 * (j + 1)
            qsl = slice(tb + 512 * j, tb + 512 * (j + 1))
            attT = attT_tiles[(4 * b + j) % 2]
            # S^T in groups of 2 k-tiles x 2 heads -> one psum tile
            for g in range(kmax // 2):
                sps = ps_s.tile([128, 2048], F32, tag="s", name=f"sps{b}_{j}_{g}")
                for i in range(2):
                    kt = 2 * g + i
                    ksl = slice(tb + 128 * kt, tb + 128 * (kt + 1))
                    for h in range(2):
                        hsl = slice(64 * h, 64 * (h + 1))
                        nc.tensor.matmul(
                            sps[:, 1024 * h + 512 * i:1024 * h + 512 * (i + 1)],
                            kT_sb[hsl, ksl], qT_sb[hsl, qsl],
                            start=True, stop=True)
                # exp; on diagonal k-tiles only the causally valid q-columns
                lo = 0
                for i in range(2):
                    kt = 2 * g + i
                    if kt >= 4 * j:
                        lo = max(lo, 128 * (kt - 4 * j) * 0)  # per-slot below
                for i in range(2):
                    kt = 2 * g + i
                    q0 = 128 * (kt - 4 * j) if kt >= 4 * j else 0
                    dst = attT[:].rearrange("p (h s) -> p h s", h=2)[
                        :, :, 512 * kt + q0:512 * (kt + 1)]
                    srcv = sps[:].rearrange("p (h s) -> p h s", h=2)[
                        :, :, 512 * i + q0:512 * (i + 1)]
                    nc.scalar.activation(dst, srcv, ActF.Exp,
                                         scale=float(1.0 / np.sqrt(D)))
            # causal masks on the 4 diagonal k-tiles
            for i in range(4):
                kt = 4 * j + i
                for h in range(2):
                    a = attT[:, 8192 * h + 512 * kt:8192 * h + 512 * (kt + 1)]
                    nc.vector.tensor_mul(a, a, mask_sb[:, 512 * i:512 * (i + 1)])
            # A @ V (ones column gives the softmax denominator in row 64)
            for h in range(2):
                yps = ps_y.tile([65, 512], F32, tag="y", name=f"yps{b}_{j}_{h}")
                for kt in range(kmax):
                    nc.tensor.matmul(
                        yps[:], v_all[:, 130 * (b * 16 + kt) + 65 * h:
                                      130 * (b * 16 + kt) + 65 * (h + 1)],
                        attT[:, 8192 * h + 512 * kt:8192 * h + 512 * (kt + 1)],
                        start=(kt == 0), stop=(kt == kmax - 1))
                ysb = sb_tmp.tile([65, 512], F32, tag="ysb", name=f"ysb{b}{j}{h}")
                nc.vector.tensor_copy(ysb[:], yps[:])
                ltmp = sb_tmp.tile([1, 512], F32, tag="ltmp", name=f"lt{b}{j}{h}")
                nc.vector.tensor_copy(ltmp[:], ysb[64:65, :])
                recf = sb_tmp.tile([1, 512], F32, tag="recf", name=f"rf{b}{j}{h}")
                nc.vector.reciprocal_approx_fast(recf[:], ltmp[:])
                rec = sb_tmp.tile([1, 512], BF, tag="rec", name=f"rc{b}{j}{h}")
                nc.vector.tensor_copy(rec[:], recf[:])
                bc = ps_mm.tile([64, 512], F32, tag="mm", name=f"bc{b}{j}{h}")
                nc.tensor.matmul(bc[:], ones_sb[:, 0:64], rec[:],
                                 start=True, stop=True)
                nc.vector.tensor_mul(
                    yT_sb[64 * h:64 * (h + 1), qsl], ysb[0:64, :], bc[:])

        # batch-0 windows, then batch-0 attention interleaved (in priority)
        # with batch-1 windows, then batch-1 attention.
        for n in range(4):
            qkv_window(n)
        for j in range(NQ):
            attention_chunk(0, j)
        for n in range(4, 8):
            qkv_window(n)
        for j in range(NQ):
            attention_chunk(1, j)

        # ---- AllToAll: head-shard -> token-shard ----
        cc_in = dram.tile([NCORES * 128, 512], BF, tag="ccin")
        cc_out = dram.tile([NCORES * 128, 512], BF, tag="ccout")
        for sh in range(NCORES):
            nc.gpsimd.dma_start(cc_in[128 * sh:128 * (sh + 1), :],
                                yT_sb[:, 512 * sh:512 * (sh + 1)])
        nc.gpsimd.collective_compute(
            "AllToAll", mybir.AluOpType.bypass,
            replica_groups=[list(range(NCORES))],
            ins=[cc_in[:]], outs=[cc_out[:]])

        y_lhs = []
        for kk in range(NK):
            t = sb_tmp.tile([128, 512], BF, tag=f"ylhs{kk}")
            nc.gpsimd.dma_start(t[:], cc_out[128 * kk:128 * (kk + 1), :])
            y_lhs.append(t)

        # wproj is only needed at the end; let its DMA fill idle mid-kernel
        # bandwidth (emitted late so it never delays the x/w/mask loads).
        wproj_sb = []
        for kk in range(NK):
            t = sb_w.tile([128, C], BF, tag=f"wproj{kk}")
            nc.gpsimd.dma_start(t[:], wproj[128 * kk:128 * (kk + 1), :])
            wproj_sb.append(t)

        # ---- output projection: out[512, 1024] ----
        for mt in range(SHARD // 128):
            pss = []
            for nn in range(C // 512):
                ps = ps_mm.tile([128, 512], F32, tag="mm", name=f"prj{mt}_{nn}")
                pss.append(ps)
            for kk in range(NK):
                for nn in range(C // 512):
                    nc.tensor.matmul(
                        pss[nn][:], y_lhs[kk][:, 128 * mt:128 * (mt + 1)],
                        wproj_sb[kk][:, 512 * nn:512 * (nn + 1)],
                        start=(kk == 0), stop=False)
            for nn in range(C // 512):
                nc.tensor.matmul(
                    pss[nn][:], ones_sb[:, 0:128],
                    brows_sb[:, 384 + 512 * nn:384 + 512 * (nn + 1)],
                    start=False, stop=True)
                o = sb_out.tile([128, 512], F32, tag="o", name=f"o{mt}_{nn}")
                nc.vector.tensor_copy(o[:], pss[nn][:])
                nc.gpsimd.dma_start(
                    out[128 * mt:128 * (mt + 1), 512 * nn:512 * (nn + 1)], o[:])

    nc.compile()
    return nc


def _host_inputs(x, W_qkv, b_qkv, W_proj, b_proj):
    """Shard/layout/cast inputs for each core."""
    bf = ml_dtypes.bfloat16
    xT = np.ascontiguousarray(
        x.reshape(TT, C).T).astype(bf)                    # [C, TT]
    wproj = W_proj.astype(bf)                             # [C, C]
    kk_idx = np.arange(128)[:, None]
    qq_idx = np.arange(512)[None, :]
    masks = np.concatenate(
        [(128 * i + kk_idx <= qq_idx) for i in range(4)],
        axis=1).astype(bf)                                # [128, 2048]
    ident = np.eye(128).astype(bf)

    in_maps = []
    for c in range(NCORES):
        h0 = HP * c * D
        cols = slice(h0, h0 + HP * D)                     # 128 cols
        wq = W_qkv[:, cols]
        wk = W_qkv[:, C:][:, cols]
        wv = W_qkv[:, 2 * C:][:, cols]
        wqkv = np.concatenate([wq, wk, wv], axis=1).astype(bf)   # [C, 384]
        brows = np.concatenate(
            [b_qkv[cols], b_qkv[C:][cols], b_qkv[2 * C:][cols], b_proj]
        )[None, :].astype(bf)                             # [1, 1408]
        in_maps.append({
            "xT": xT, "wqkv": wqkv, "wproj": wproj,
            "masks": masks, "brows": brows, "ident": ident,
        })
    return in_maps


def kernel(x, W_qkv, b_qkv, W_proj, b_proj):
    global LAST_EXEC_TIME_NS, LAST_RESULTS
    x = np.asarray(x, dtype=np.float32)
    W_qkv = np.asarray(W_qkv, dtype=np.float32)
    b_qkv = np.asarray(b_qkv, dtype=np.float32)
    W_proj = np.asarray(W_proj, dtype=np.float32)
    b_proj = np.asarray(b_proj, dtype=np.float32)

    if "nc" not in _CACHE:
        _CACHE["nc"] = build_nc()
    nc = _CACHE["nc"]

    in_maps = _host_inputs(x, W_qkv, b_qkv, W_proj, b_proj)
    trace = os.environ.get("TRN_KERNEL_TRACE", "0") == "1"
    kw = {}
    if os.environ.get("TRN_KERNEL_TRACE_ALL", "0") == "1":
        kw["trace_cores"] = list(range(NCORES))
    res = run_bass_kernel_spmd(nc, in_maps, core_ids=list(range(NCORES)),
                               trace=trace, **kw)
    LAST_EXEC_TIME_NS = res.exec_time_ns
    LAST_RESULTS = res
    out = np.concatenate([res.results[c]["out"] for c in range(NCORES)],
                         axis=0)
    return out.reshape(B, T, C).astype(np.float32)


# revision 14
# speedup vs baseline: 1.2055x; 1.1116x over previous
"""Causal self-attention (B=2, T=2048, C=1024, H=16) on 8 TRN2 NeuronCores.

Sharding (Megatron-style, per the hint): each core owns one PAIR of heads
(2c, 2c+1) for BOTH batches.  Column-sharded W_qkv produces qT/kT/vT in
[feature, token] layout (the host feeds x pre-transposed so contraction is
always over partitions); v is re-laid-out to natural [token, d] via PE
transposes.  Attention computes S^T = k q^T with the two heads row-packed
in the PE array (K=64 each at partition offsets 0/64), exp on ACT with
the 1/sqrt(D) scale folded in (ACT runs exp ONLY, so its spline table is
loaded once), causal masking via host-precomputed multiplicative masks on
the diagonal tiles of each q-chunk, and A@V with a ones-column appended
to v (M=65) so the softmax denominators fall out of the same matmul.
All biases are added with K=1 rank-1 matmuls into PSUM; all PSUM->SBUF
copies run on DVE.  An 8-core AllToAll swaps head-shards for
token-shards, after which each core computes its [512, 1024] slab of the
output projection with the full (replicated) W_proj.  The host only
shards/transposes/casts inputs and concatenates the 8 output slabs.

Compute dtype bf16 (f32 accumulation in PSUM); I/O f32.
"""

import os
import sys
import types

import numpy as np

if "/opt/trn_rl_repo" not in sys.path:
    sys.path.insert(0, "/opt/trn_rl_repo")

# antenv.axon_hooks is missing on this image; shim it so trace=True can
# capture NTFF profiles (used by test harnesses; harmless otherwise).
if "antenv.axon_hooks" not in sys.modules:
    _hooks_mod = types.ModuleType("antenv.axon_hooks")
    _holder = {"hook": None}
    _hooks_mod.set_axon_ntff_profile_hook = lambda h: _holder.__setitem__("hook", h)
    _hooks_mod.get_axon_ntff_profile_hook = lambda: _holder["hook"]
    sys.modules["antenv.axon_hooks"] = _hooks_mod
    try:
        from trn_agent_boot.trn_boot import _ntff_profile_via_ctypes

        _hooks_mod.set_axon_ntff_profile_hook(
            _ntff_profile_via_ctypes("/opt/axon/libaxon_pjrt.so")
        )
    except Exception:
        pass

import ml_dtypes
from contextlib import ExitStack

import concourse.bacc as bacc
import concourse.tile as tile
from concourse import mybir
from concourse.bass_utils import run_bass_kernel_spmd

B, T, C, H = 2, 2048, 1024, 16
D = C // H          # 64
NCORES = 8
HP = 2              # heads per core
TT = B * T          # 4096 global (b, t) rows
NK = C // 128       # 8 contraction tiles over features
NW = TT // 512      # 8 token windows
NQ = T // 512       # 4 q-chunks per batch
SHARD = TT // NCORES  # 512 output rows per core

F32 = mybir.dt.float32
BF = mybir.dt.bfloat16

ActF = mybir.ActivationFunctionType

_CACHE = {}

LAST_EXEC_TIME_NS = None
LAST_RESULTS = None


def build_nc():
    nc = bacc.Bacc("TRN2", target_bir_lowering=False, debug=False,
                   num_devices=NCORES)

    xT = nc.declare_dram_parameter("xT", [C, TT], BF, isOutput=False)
    wqkv = nc.declare_dram_parameter("wqkv", [C, 3 * 128], BF, isOutput=False)
    wproj = nc.declare_dram_parameter("wproj", [C, C], BF, isOutput=False)
    masks = nc.declare_dram_parameter("masks", [128, 4 * 512], BF, isOutput=False)
    brows = nc.declare_dram_parameter("brows", [1, 3 * 128 + C], BF, isOutput=False)
    ident = nc.declare_dram_parameter("ident", [128, 128], BF, isOutput=False)
    out = nc.declare_dram_parameter("out", [SHARD, C], F32, isOutput=True)

    with tile.TileContext(nc) as tc, ExitStack() as ctx:
        sb_x = ctx.enter_context(tc.tile_pool(name="sb_x", bufs=2))
        sb_w = ctx.enter_context(tc.tile_pool(name="sb_w", bufs=1))
        sb_qk = ctx.enter_context(tc.tile_pool(name="sb_qk", bufs=1))
        sb_v = ctx.enter_context(tc.tile_pool(name="sb_v", bufs=1))
        sb_att = ctx.enter_context(tc.tile_pool(name="sb_att", bufs=1))
        sb_y = ctx.enter_context(tc.tile_pool(name="sb_y", bufs=1))
        sb_tmp = ctx.enter_context(tc.tile_pool(name="sb_tmp", bufs=2))
        sb_out = ctx.enter_context(tc.tile_pool(name="sb_out", bufs=2))
        ps_mm = ctx.enter_context(tc.tile_pool(name="ps_mm", bufs=2, space="PSUM"))
        ps_s = ctx.enter_context(tc.tile_pool(name="ps_s", bufs=1, space="PSUM"))
        ps_y = ctx.enter_context(tc.tile_pool(name="ps_y", bufs=2, space="PSUM"))
        dram = ctx.enter_context(tc.tile_pool(name="dram", bufs=1, space="DRAM"))

        # ---- small loads first (weights, masks, biases) ----
        wqkv_sb = []
        for kk in range(NK):
            t = sb_w.tile([128, 3 * 128], BF, tag=f"wqkv{kk}")
            nc.sync.dma_start(t[:], wqkv[128 * kk:128 * (kk + 1), :])
            wqkv_sb.append(t)
        mask_sb = sb_w.tile([128, 4 * 512], BF, tag="mask")
        nc.sync.dma_start(mask_sb[:], masks[:])
        brows_sb = sb_w.tile([1, 3 * 128 + C], BF, tag="brows")
        nc.gpsimd.dma_start(brows_sb[:], brows[:])
        ident_sb = sb_w.tile([128, 128], BF, tag="ident")
        nc.gpsimd.dma_start(ident_sb[:], ident[:])

        ones_sb = sb_w.tile([1, 512], BF, tag="ones")
        nc.vector.memset(ones_sb[:], 1.0)

        # qT/kT/vT: [128 (=2 heads x 64 features), 4096 tokens]
        qT_sb = sb_qk.tile([128, TT], BF, tag="qT")
        kT_sb = sb_qk.tile([128, TT], BF, tag="kT")
        vT_sb = sb_qk.tile([128, TT], BF, tag="vT")
        dests = [qT_sb, kT_sb, vT_sb]
        # v natural: [128 tokens, 32 tiles x 130] = [vA(64) | 1 | vB(64) | 1]
        v_all = sb_v.tile([128, (TT // 128) * 130], BF, tag="v")
        v_sb = [v_all[:, 130 * tt:130 * (tt + 1)] for tt in range(TT // 128)]

        # ---- QKV projection, streamed over token windows ----
        def qkv_window(n):
            xw = []
            for kk in range(NK):
                t = sb_x.tile([128, 512], BF, tag=f"xw{kk}", name=f"xw{kk}_{n}")
                nc.sync.dma_start(t[:], xT[128 * kk:128 * (kk + 1),
                                             512 * n:512 * (n + 1)])
                xw.append(t)
            for m in range(3):
                ps = ps_mm.tile([128, 512], F32, tag="mm", name=f"qkvps{n}_{m}")
                for kk in range(NK):
                    nc.tensor.matmul(
                        ps[:], wqkv_sb[kk][:, 128 * m:128 * (m + 1)], xw[kk][:],
                        start=(kk == 0), stop=False)
                # bias over partitions: ps[p, t] += b[p] * 1
                nc.tensor.matmul(ps[:], brows_sb[:, 128 * m:128 * (m + 1)],
                                 ones_sb[:], start=False, stop=True)
                nc.vector.tensor_copy(dests[m][:, 512 * n:512 * (n + 1)], ps[:])
            # transpose v window into natural layout
            for tt in range(4 * n, 4 * (n + 1)):
                tp = ps_mm.tile([128, 128], BF, tag="mm", name=f"vtp{tt}")
                nc.tensor.transpose(tp[:], vT_sb[:, 128 * tt:128 * (tt + 1)],
                                    ident_sb[:])
                vt = v_sb[tt]
                nc.vector.memset(vt, 1.0)
                nc.vector.tensor_copy(
                    vt.rearrange("p (h c) -> p h c", c=65)[:, :, 0:64],
                    tp[:].rearrange("p (h c) -> p h c", c=64))

        # ---- attention ----
        yT_sb = sb_y.tile([128, TT], BF, tag="yT")
        # two attT tiles, alternated across chunks; memset once so that the
        # exp-skipped (causally invalid) columns of diagonal tiles hold
        # finite stale data for the mask-multiply to zero.
        attT_tiles = [
            sb_att.tile([128, 2 * 16 * 512], BF, tag=f"attT{i}", name=f"attT{i}")
            for i in range(2)]

        def attention_chunk(b, j):
            tb = b * T
            kmax = 4 * (j + 1)
            qsl = slice(tb + 512 * j, tb + 512 * (j + 1))
            attT = attT_tiles[(4 * b + j) % 2]
            # S^T in groups of 2 k-tiles x 2 heads -> one psum tile
            for g in range(kmax // 2):
                sps = ps_s.tile([128, 2048], F32, tag="s", name=f"sps{b}_{j}_{g}")
                for i in range(2):
                    kt = 2 * g + i
                    ksl = slice(tb + 128 * kt, tb + 128 * (kt + 1))
                    for h in range(2):
                        hsl = slice(64 * h, 64 * (h + 1))
                        nc.tensor.matmul(
                            sps[:, 1024 * h + 512 * i:1024 * h + 512 * (i + 1)],
                            kT_sb[hsl, ksl], qT_sb[hsl, qsl],
                            start=True, stop=True)
                # exp over the whole group (both heads)
                dst = attT[:].rearrange("p (h s) -> p h s", h=2)[
                    :, :, 512 * 2 * g:512 * 2 * (g + 1)]
                nc.scalar.activation(dst, sps[:].rearrange(
                    "p (h s) -> p h s", h=2), ActF.Exp,
                    scale=float(1.0 / np.sqrt(D)))
            # causal masks on the 4 diagonal k-tiles
            for i in range(4):
                kt = 4 * j + i
                for h in range(2):
                    a = attT[:, 8192 * h + 512 * kt:8192 * h + 512 * (kt + 1)]
                    nc.vector.tensor_mul(a, a, mask_sb[:, 512 * i:512 * (i + 1)])
            # A @ V (ones column gives the softmax denominator in row 64)
            for h in range(2):
                yps = ps_y.tile([65, 512], F32, tag="y", name=f"yps{b}_{j}_{h}")
                for kt in range(kmax):
                    nc.tensor.matmul(
                        yps[:], v_all[:, 130 * (b * 16 + kt) + 65 * h:
                                      130 * (b * 16 + kt) + 65 * (h + 1)],
                        attT[:, 8192 * h + 512 * kt:8192 * h + 512 * (kt + 1)],
                        start=(kt == 0), stop=(kt == kmax - 1))
                ysb = sb_tmp.tile([65, 512], F32, tag="ysb", name=f"ysb{b}{j}{h}")
                nc.vector.tensor_copy(ysb[:], yps[:])
                ltmp = sb_tmp.tile([1, 512], F32, tag="ltmp", name=f"lt{b}{j}{h}")
                nc.vector.tensor_copy(ltmp[:], ysb[64:65, :])
                recf = sb_tmp.tile([1, 512], F32, tag="recf", name=f"rf{b}{j}{h}")
                nc.vector.reciprocal_approx_fast(recf[:], ltmp[:])
                rec = sb_tmp.tile([1, 512], BF, tag="rec", name=f"rc{b}{j}{h}")
                nc.vector.tensor_copy(rec[:], recf[:])
                bc = ps_mm.tile([64, 512], F32, tag="mm", name=f"bc{b}{j}{h}")
                nc.tensor.matmul(bc[:], ones_sb[:, 0:64], rec[:],
                                 start=True, stop=True)
                nc.vector.tensor_mul(
                    yT_sb[64 * h:64 * (h + 1), qsl], ysb[0:64, :], bc[:])

        # batch-0 windows, then batch-0 attention interleaved (in priority)
        # with batch-1 windows, then batch-1 attention.
        for n in range(4):
            qkv_window(n)
        for j in range(NQ):
            attention_chunk(0, j)
        for n in range(4, 8):
            qkv_window(n)
        for j in range(NQ):
            attention_chunk(1, j)

        # ---- AllToAll: head-shard -> token-shard ----
        cc_in = dram.tile([NCORES * 128, 512], BF, tag="ccin")
        cc_out = dram.tile([NCORES * 128, 512], BF, tag="ccout")
        for sh in range(NCORES):
            nc.gpsimd.dma_start(cc_in[128 * sh:128 * (sh + 1), :],
                                yT_sb[:, 512 * sh:512 * (sh + 1)])
        nc.gpsimd.collective_compute(
            "AllToAll", mybir.AluOpType.bypass,
            replica_groups=[list(range(NCORES))],
            ins=[cc_in[:]], outs=[cc_out[:]])

        y_lhs = []
        for kk in range(NK):
            t = sb_tmp.tile([128, 512], BF, tag=f"ylhs{kk}")
            nc.sync.dma_start(t[:], cc_out[128 * kk:128 * (kk + 1), :])
            y_lhs.append(t)

        # wproj is only needed at the end; let its DMA fill idle mid-kernel
        # bandwidth (emitted late so it never delays the x/w/mask loads).
        wproj_sb = []
        for kk in range(NK):
            t = sb_w.tile([128, C], BF, tag=f"wproj{kk}")
            nc.sync.dma_start(t[:], wproj[128 * kk:128 * (kk + 1), :])
            wproj_sb.append(t)

        # ---- output projection: out[512, 1024] ----
        for mt in range(SHARD // 128):
            pss = []
            for nn in range(C // 512):
                ps = ps_mm.tile([128, 512], F32, tag="mm", name=f"prj{mt}_{nn}")
                pss.append(ps)
            for kk in range(NK):
                for nn in range(C // 512):
                    nc.tensor.matmul(
                        pss[nn][:], y_lhs[kk][:, 128 * mt:128 * (mt + 1)],
                        wproj_sb[kk][:, 512 * nn:512 * (nn + 1)],
                        start=(kk == 0), stop=False)
            for nn in range(C // 512):
                nc.tensor.matmul(
                    pss[nn][:], ones_sb[:, 0:128],
                    brows_sb[:, 384 + 512 * nn:384 + 512 * (nn + 1)],
                    start=False, stop=True)
                o = sb_out.tile([128, 512], F32, tag="o", name=f"o{mt}_{nn}")
                nc.vector.tensor_copy(o[:], pss[nn][:])
                nc.sync.dma_start(
                    out[128 * mt:128 * (mt + 1), 512 * nn:512 * (nn + 1)], o[:])

    nc.compile()
    return nc


def _host_inputs(x, W_qkv, b_qkv, W_proj, b_proj):
    """Shard/layout/cast inputs for each core."""
    bf = ml_dtypes.bfloat16
    xT = np.ascontiguousarray(
        x.reshape(TT, C).T).astype(bf)                    # [C, TT]
    wproj = W_proj.astype(bf)                             # [C, C]
    kk_idx = np.arange(128)[:, None]
    qq_idx = np.arange(512)[None, :]
    masks = np.concatenate(
        [(128 * i + kk_idx <= qq_idx) for i in range(4)],
        axis=1).astype(bf)                                # [128, 2048]
    ident = np.eye(128).astype(bf)

    in_maps = []
    for c in range(NCORES):
        h0 = HP * c * D
        cols = slice(h0, h0 + HP * D)                     # 128 cols
        wq = W_qkv[:, cols]
        wk = W_qkv[:, C:][:, cols]
        wv = W_qkv[:, 2 * C:][:, cols]
        wqkv = np.concatenate([wq, wk, wv], axis=1).astype(bf)   # [C, 384]
        brows = np.concatenate(
            [b_qkv[cols], b_qkv[C:][cols], b_qkv[2 * C:][cols], b_proj]
        )[None, :].astype(bf)                             # [1, 1408]
        in_maps.append({
            "xT": xT, "wqkv": wqkv, "wproj": wproj,
            "masks": masks, "brows": brows, "ident": ident,
        })
    return in_maps


def kernel(x, W_qkv, b_qkv, W_proj, b_proj):
    global LAST_EXEC_TIME_NS, LAST_RESULTS
    x = np.asarray(x, dtype=np.float32)
    W_qkv = np.asarray(W_qkv, dtype=np.float32)
    b_qkv = np.asarray(b_qkv, dtype=np.float32)
    W_proj = np.asarray(W_proj, dtype=np.float32)
    b_proj = np.asarray(b_proj, dtype=np.float32)

    if "nc" not in _CACHE:
        _CACHE["nc"] = build_nc()
    nc = _CACHE["nc"]

    in_maps = _host_inputs(x, W_qkv, b_qkv, W_proj, b_proj)
    trace = os.environ.get("TRN_KERNEL_TRACE", "0") == "1"
    kw = {}
    if os.environ.get("TRN_KERNEL_TRACE_ALL", "0") == "1":
        kw["trace_cores"] = list(range(NCORES))
    res = run_bass_kernel_spmd(nc, in_maps, core_ids=list(range(NCORES)),
                               trace=trace, **kw)
    LAST_EXEC_TIME_NS = res.exec_time_ns
    LAST_RESULTS = res
    out = np.concatenate([res.results[c]["out"] for c in range(NCORES)],
                         axis=0)
    return out.reshape(B, T, C).astype(np.float32)
